# revision 9
# baseline (speedup 1.0000x reference)
"""Multi-head attention (B=2, S=2048, D=1024, H=16) on 8 Trainium2 NeuronCores.

Sharding: core = b*4 + hg  (b = batch, hg = head-group of 4 heads).

Fully-pipelined single-pass structure per core (vs the phase-serial v0):
  - X/W DMAs are chunked by 512 s-columns and ordered so k-proj chunk 0 and
    q-proj chunk 0 can start ~3us in; first score matmul at ~10us.
  - Attention runs in 4 sweeps (sqc = 512 q-columns each), skc inner
    (16 chunks of 128 k-rows), paced by back-to-back scalar-engine EXPs
    (the irreducible bottleneck: 16.8M exps/core at 1 elem/lane/cycle).
  - v-proj/k-proj chunks interleave just-in-time inside sweep 0;
    q-proj(sqc+1) and out-proj(sqc-1) interleave inside later sweeps, so
    almost no tensor work remains outside the ACT-bound span.
  - PV is col-packed: per head-pair one span of two concurrent matmuls at
    tile_position (0,0)/(0,64) (full 128 PE columns vs 65 in v0). Rowsums
    come from four concurrent M=1 ones-matmuls col-tiled at (0,32h).
  - Normalization: DVE reciprocal_approx_fast on the rowsum rows + gpsimd
    broadcast-DMA + DVE multiply straight into attnT. The scalar engine
    does nothing but EXP (single activation table set).

All matmul inputs bf16 (PSUM accumulation f32). Host adds bo and sums the
4 head-group partials per batch.
"""

import numpy as np
import ml_dtypes

import concourse.bacc as bacc
import concourse.mybir as mybir
import concourse.tile as tile
from concourse.bass_utils import run_bass_kernel_spmd

BF16 = mybir.dt.bfloat16
F32 = mybir.dt.float32
AF = mybir.ActivationFunctionType
ALU = mybir.AluOpType

B, S, D = 2, 2048, 1024
H = 16
DK = 64
NCORES = 8
HG = 4  # head groups
HPG = 4  # heads per group
GO = HPG * DK  # 256 group output width
NIC = D // 128  # 8 contraction chunks
NSC = S // 128  # 16 sk chunks
NSQ = S // 512  # 4 sq chunks

_NC = None


def _emit(nc, tc, io):
    xqT, xkT, xvT, wqT, wkT, wvT, woT, bqk, outT = (
        io["xqT"], io["xkT"], io["xvT"], io["wqT"], io["wkT"], io["wvT"],
        io["woT"], io["bqk"], io["outT"],
    )

    with (
        tc.tile_pool(name="wp", bufs=1) as wp,
        tc.tile_pool(name="xp", bufs=1) as xp,
        tc.tile_pool(name="pp", bufs=1) as pp,
        tc.tile_pool(name="pt", bufs=4) as ptp,
        tc.tile_pool(name="rr", bufs=2) as rrp,
        tc.tile_pool(name="rb", bufs=6) as rbp,
        tc.tile_pool(name="fo", bufs=6) as fop,
        tc.tile_pool(name="psS", bufs=2, space="PSUM") as psS,
        tc.tile_pool(name="psPV", bufs=2, space="PSUM") as psPV,
        tc.tile_pool(name="psRS", bufs=1, space="PSUM") as psRS,
        tc.tile_pool(name="psX", bufs=1, space="PSUM") as psX,
        tc.tile_pool(name="dr", bufs=4, space="DRAM") as drp,
    ):
        # ---------------- tiles ----------------
        bqk_t = wp.tile([128, 4], F32, name="bqk", tag="bqk")
        wk = [wp.tile([128, GO], BF16, name=f"wk{i}", tag=f"wk{i}") for i in range(NIC)]
        wq = [wp.tile([128, GO], BF16, name=f"wq{i}", tag=f"wq{i}") for i in range(NIC)]
        wv = [wp.tile([128, GO], BF16, name=f"wv{i}", tag=f"wv{i}") for i in range(NIC)]
        wv_b = wp.tile([1, GO], BF16, name="wvb", tag="wvb")
        wo = [wp.tile([128, D], BF16, name=f"wo{o}", tag=f"wo{o}") for o in range(2)]
        ones_col = wp.tile([128, 1], BF16, name="ones", tag="ones")

        # x inputs as separate [128,512] tiles per (ic, col-chunk) for clean deps
        xk = [[xp.tile([128, 512], BF16, name=f"xk{i}_{c}", tag=f"xk{i}_{c}")
               for c in range(4)] for i in range(NIC)]
        xq = [[xp.tile([128, 512], BF16, name=f"xq{i}_{c}", tag=f"xq{i}_{c}")
               for c in range(4)] for i in range(NIC)]
        xv = [[xp.tile([128, 512], BF16, name=f"xv{i}_{c}", tag=f"xv{i}_{c}")
               for c in range(4)] for i in range(NIC)]
        xv_ones = [xp.tile([1, 512], BF16, name=f"xvo{c}", tag=f"xvo{c}")
                   for c in range(4)]

        # projected tensors, chunked [128, 512] per (oc, c)
        kT = [[pp.tile([128, 512], BF16, name=f"kT{o}_{c}", tag=f"kT{o}_{c}")
               for c in range(4)] for o in range(2)]
        qT = [[pp.tile([128, 512], BF16, name=f"qT{o}_{c}", tag=f"qT{o}_{c}")
               for c in range(4)] for o in range(2)]
        v = [pp.tile([128, GO], BF16, name=f"v{k}", tag=f"v{k}") for k in range(NSC)]
        attnT = [[pp.tile([128, 512], BF16, name=f"at{o}_{c}", tag=f"at{o}_{c}")
                  for c in range(4)] for o in range(2)]

        nc.vector.memset(ones_col[:], 1.0)

        # ---------------- DMAs, chunk-ordered ----------------
        nc.sync.dma_start(bqk_t[:], bqk[:])
        for i in range(NIC):
            nc.sync.dma_start(wk[i][:], wkT[128 * i:128 * i + 128, :])
        for i in range(NIC):
            nc.gpsimd.dma_start(xk[i][0][:], xkT[128 * i:128 * i + 128, 0:512])
        for i in range(NIC):
            nc.sync.dma_start(wq[i][:], wqT[128 * i:128 * i + 128, :])
        for i in range(NIC):
            nc.gpsimd.dma_start(xq[i][0][:], xqT[128 * i:128 * i + 128, 0:512])
        for i in range(NIC):
            nc.sync.dma_start(wv[i][:], wvT[128 * i:128 * i + 128, :])
        nc.sync.dma_start(wv_b[:], wvT[D:D + 1, :])
        for i in range(NIC):
            nc.gpsimd.dma_start(xv[i][0][:], xvT[128 * i:128 * i + 128, 0:512])
        nc.sync.dma_start(xv_ones[0][:], xvT[D:D + 1, 0:512])
        # remaining chunks: xk/xv rounds first (consumed inside sweep 0),
        # xq chunks after (consumed from late sweep 0 onward)
        for c in range(1, 4):
            for i in range(NIC):
                eng = nc.gpsimd if i % 2 == 0 else nc.sync
                eng.dma_start(xk[i][c][:], xkT[128 * i:128 * i + 128, 512 * c:512 * c + 512])
            for i in range(NIC):
                eng = nc.gpsimd if i % 2 == 1 else nc.sync
                eng.dma_start(xv[i][c][:], xvT[128 * i:128 * i + 128, 512 * c:512 * c + 512])
            nc.sync.dma_start(xv_ones[c][:], xvT[D:D + 1, 512 * c:512 * c + 512])
        for c in range(1, 4):
            for i in range(NIC):
                eng = nc.gpsimd if i % 2 == 0 else nc.sync
                eng.dma_start(xq[i][c][:], xqT[128 * i:128 * i + 128, 512 * c:512 * c + 512])
        for o in range(2):
            nc.sync.dma_start(wo[o][:], woT[128 * o:128 * o + 128, :])

        # ---------------- projection helpers ----------------
        def kqproj_chunk_psS(out_kq, w, x, bias_col0, c):
            # prefix-only: both oc halves in one [128,1024] scores-pool tile
            ps = psS.tile([128, 1024], F32, name="s", tag="s")
            for oc in range(2):
                for ic in range(NIC):
                    nc.tensor.matmul(
                        ps[:, 512 * oc:512 * oc + 512],
                        w[ic][:, 128 * oc:128 * oc + 128],
                        x[ic][c][:],
                        start=(ic == 0),
                        stop=(ic == NIC - 1),
                    )
            for oc in range(2):
                nc.vector.tensor_scalar(
                    out_kq[oc][c][:], ps[:, 512 * oc:512 * oc + 512],
                    bqk_t[:, bias_col0 + oc:bias_col0 + oc + 1], None,
                    op0=ALU.add,
                )

        def kqproj_chunk_aux(out_kq, w, x, bias_col0, c):
            # steady-state: one oc at a time through the 1-bank aux pool
            for oc in range(2):
                ps = psX.tile([128, 512], F32, name="x", tag="x")
                for ic in range(NIC):
                    nc.tensor.matmul(
                        ps[:],
                        w[ic][:, 128 * oc:128 * oc + 128],
                        x[ic][c][:],
                        start=(ic == 0),
                        stop=(ic == NIC - 1),
                    )
                nc.vector.tensor_scalar(
                    out_kq[oc][c][:], ps[:],
                    bqk_t[:, bias_col0 + oc:bias_col0 + oc + 1], None,
                    op0=ALU.add,
                )

        def vproj_chunk(k):
            c, j = k // 4, k % 4
            ps = psX.tile([128, 512], F32, name="x", tag="x")
            for ic in range(NIC):
                nc.tensor.matmul(
                    ps[:, 0:GO],
                    xv[ic][c][:, 128 * j:128 * j + 128],
                    wv[ic][:],
                    start=(ic == 0),
                    stop=False,
                )
            nc.tensor.matmul(
                ps[:, 0:GO],
                xv_ones[c][:, 128 * j:128 * j + 128],
                wv_b[:],
                start=False,
                stop=True,
            )
            nc.vector.tensor_copy(v[k][:], ps[:, 0:GO])

        def fproj(sqc):
            # output projection for one finished sq chunk
            for mc in range(D // 128):
                fac = psX.tile([128, 512], F32, name="x", tag="x")
                for oc in range(2):
                    nc.tensor.matmul(
                        fac[:],
                        wo[oc][:, 128 * mc:128 * mc + 128],
                        attnT[oc][sqc][:],
                        start=(oc == 0),
                        stop=(oc == 1),
                    )
                fo_ = fop.tile([128, 512], BF16, name="fo", tag="fo")
                nc.vector.tensor_copy(fo_[:], fac[:])
                eng = nc.sync if mc % 2 == 0 else nc.gpsimd
                eng.dma_start(
                    outT[128 * mc:128 * mc + 128, 512 * sqc:512 * sqc + 512],
                    fo_[:],
                )

        # ---------------- attention ----------------
        def normalize(sqc, accP, rs):
            # one full-tile reciprocal: the custom-DVE op misbehaves at
            # nonzero base partitions, so compute all 128 rows (unused rows
            # hold garbage that is never read)
            rr = rrp.tile([128, 512], F32, name="rr", tag="rr")
            nc.vector.reciprocal_approx_fast(rr[:], rs[:])
            # per-pair [128,512] broadcast tile (row halves = the two heads)
            # so the multiply's lanes line up with accP/attnT partitions
            for p in range(2):
                rb = rbp.tile([128, 512], F32, name="rb", tag="rb")
                for sub in range(2):
                    h = 2 * p + sub
                    rd = drp.tile([1, 512], F32, name="rd", tag="rd")
                    nc.sync.dma_start(rd[:], rr[32 * h:32 * h + 1, :])
                    nc.gpsimd.dma_start(
                        rb[64 * sub:64 * sub + 64, :],
                        rd.to_broadcast([64, 512]))
                nc.vector.tensor_mul(attnT[p][sqc][:], accP[p][:], rb[:])

        # prefix projections (chunk 0 of k and q)
        kqproj_chunk_psS(kT, wk, xk, 2, 0)
        kqproj_chunk_psS(qT, wq, xq, 0, 0)

        for sqc in range(NSQ):
            accP = [psPV.tile([128, 512], F32, name="pv", tag="pv")
                    for _ in range(2)]
            rs = psRS.tile([128, 512], F32, name="rs", tag="rs")

            def emit_pv_rs(prev):
                pTs, k = prev
                for p in range(2):
                    nc.tensor.matmul(
                        accP[p][0:64, :],
                        v[k][:, 128 * p:128 * p + 64],
                        pTs[p][:, 0:512],
                        start=(k == 0),
                        stop=(k == NSC - 1),
                        tile_position=(0, 0),
                    )
                    nc.tensor.matmul(
                        accP[p][64:128, :],
                        v[k][:, 128 * p + 64:128 * p + 128],
                        pTs[p][:, 512:1024],
                        start=(k == 0),
                        stop=(k == NSC - 1),
                        tile_position=(0, 64),
                    )
                for h in range(HPG):
                    nc.tensor.matmul(
                        rs[32 * h:32 * h + 1, :],
                        ones_col[:, 0:1],
                        pTs[h // 2][:, 512 * (h % 2):512 * (h % 2) + 512],
                        start=(k == 0),
                        stop=(k == NSC - 1),
                        tile_position=(0, 32 * h),
                    )

            prev = None
            for skc in range(NSC):
                # interleaved producer work (timed against DMA arrival)
                if sqc == 0:
                    vproj_chunk(skc)
                    if skc in (2, 7, 9):
                        kqproj_chunk_aux(kT, wk, xk, 2, {2: 1, 7: 2, 9: 3}[skc])
                    if skc == 13:
                        kqproj_chunk_aux(qT, wq, xq, 0, 1)
                else:
                    if sqc == 1 and skc == 2:
                        kqproj_chunk_aux(qT, wq, xq, 0, 2)
                    if sqc == 1 and skc == 12:
                        kqproj_chunk_aux(qT, wq, xq, 0, 3)
                    if skc == 8:
                        fproj(sqc - 1)

                kc, kj = skc // 4, skc % 4
                pTs = []
                for p in range(2):
                    ps_ = psS.tile([128, 1024], F32, name="s", tag="s")
                    for sub in range(2):
                        nc.tensor.matmul(
                            ps_[:, 512 * sub:512 * sub + 512],
                            kT[p][kc][64 * sub:64 * sub + 64,
                                      128 * kj:128 * kj + 128],
                            qT[p][sqc][64 * sub:64 * sub + 64, :],
                            start=True,
                            stop=True,
                            tile_position=(64 * sub, 0),
                        )
                    pT_ = ptp.tile([128, 1024], BF16, name="pT", tag="pT")
                    nc.scalar.activation(pT_[:], ps_[:], AF.Exp, scale=0.125)
                    pTs.append(pT_)

                # 1-step software pipeline: PV/RS of step k-1 land between
                # this step's scores and the next step's, so the PE never
                # waits on the scalar engine's exp
                if prev is not None:
                    emit_pv_rs(prev)
                prev = (pTs, skc)
            emit_pv_rs(prev)

            normalize(sqc, accP, rs)

        fproj(NSQ - 1)


def build_nc():
    nc = bacc.Bacc("TRN2", target_bir_lowering=False, debug=False,
                   num_devices=NCORES)
    io = {
        "xqT": nc.dram_tensor("xqT", [D, S], BF16, kind="ExternalInput").ap(),
        "xkT": nc.dram_tensor("xkT", [D, S], BF16, kind="ExternalInput").ap(),
        "xvT": nc.dram_tensor("xvT", [D + 1, S], BF16, kind="ExternalInput").ap(),
        "wqT": nc.dram_tensor("wqT", [D, GO], BF16, kind="ExternalInput").ap(),
        "wkT": nc.dram_tensor("wkT", [D, GO], BF16, kind="ExternalInput").ap(),
        "wvT": nc.dram_tensor("wvT", [D + 1, GO], BF16, kind="ExternalInput").ap(),
        "woT": nc.dram_tensor("woT", [GO, D], BF16, kind="ExternalInput").ap(),
        "bqk": nc.dram_tensor("bqk", [128, 4], F32, kind="ExternalInput").ap(),
        "outT": nc.dram_tensor("outT", [D, S], BF16, kind="ExternalOutput").ap(),
    }
    with tile.TileContext(nc) as tc:
        _emit(nc, tc, io)
    nc.compile()
    return nc


def get_nc():
    global _NC
    if _NC is None:
        _NC = build_nc()
    return _NC


def shard_inputs(Q, K, V, Wq, bq, Wk, bk, Wv, bv, Wo, bo):
    bf = ml_dtypes.bfloat16
    ones = np.ones((1, S), np.float32)
    in_maps = []
    for core in range(NCORES):
        b, hg = core // HG, core % HG
        rows = slice(GO * hg, GO * hg + GO)
        bq_g, bk_g, bv_g = bq[rows], bk[rows], bv[rows]
        bqk_t = np.stack(
            [bq_g[0:128], bq_g[128:256], bk_g[0:128], bk_g[128:256]], axis=1
        ).astype(np.float32)
        in_maps.append({
            "xqT": np.ascontiguousarray(Q[b].T).astype(bf),
            "xkT": np.ascontiguousarray(K[b].T).astype(bf),
            "xvT": np.concatenate([V[b].T, ones], 0).astype(bf),
            "wqT": np.ascontiguousarray(Wq[rows].T).astype(bf),
            "wkT": np.ascontiguousarray(Wk[rows].T).astype(bf),
            "wvT": np.concatenate([Wv[rows].T, bv_g[None, :]], 0).astype(bf),
            "woT": np.ascontiguousarray(Wo[:, rows].T).astype(bf),
            "bqk": bqk_t,
        })
    return in_maps


def kernel(**inputs):
    args = {k: np.asarray(v) for k, v in inputs.items()}
    nc = get_nc()
    in_maps = shard_inputs(
        args["Q"], args["K"], args["V"], args["Wq"], args["bq"], args["Wk"],
        args["bk"], args["Wv"], args["bv"], args["Wo"], args["bo"],
    )
    res = run_bass_kernel_spmd(nc, in_maps, list(range(NCORES)))
    out = np.zeros((B, S, D), np.float32)
    for core in range(NCORES):
        out[core // HG] += res.results[core]["outT"].astype(np.float32).T
    out += args["bo"].astype(np.float32)
    return out
